# revision 1
# baseline (speedup 1.0000x reference)
"""GRU cell on 8 Trainium2 NeuronCores.

Reference computation (B=65536, D=256):
    z = sigmoid(x@Wz + h@Uz + bz)
    r = sigmoid(x@Wr + h@Ur + br)
    h_hat = tanh(x@Wh + (r*h)@Uh + bh)
    h_t = z*h + (1-z)*h_hat  ; returns (h_t, h_t)

Strategy: data-parallel over the batch dim (8 shards of 8192 rows).
The host pre-transposes each shard to [256, B_shard] so every on-chip
tensor lives in [hidden, batch] layout: the contraction dim of all six
GEMMs is then the SBUF partition dim with no on-chip transposes at all,
biases become per-partition ACT bias vectors, and the elementwise gate
math runs in the same layout the matmuls produce.  Matmul operands are
bitcast to float32r (full-rate PE mode for fp32 data).
"""

import os
import sys

for _p in ("/opt/trn_rl_repo", "/root/.axon_site/_ro/trn_rl_repo"):
    if os.path.isdir(_p) and _p not in sys.path:
        sys.path.append(_p)

import numpy as np

B = 65536
D = 256
N_CORES = 8
S = B // N_CORES  # batch rows per core
CH = 512  # batch columns per chunk (one PSUM bank of fp32)

_WNAMES = ("Wz", "Uz", "Wr", "Ur", "Wh", "Uh")
_BNAMES = ("bz", "br", "bh")


def build_nc(s=S, mm_dtype_name=None, ch=CH):
    """Build + compile the per-core Bass program for a shard of s rows."""
    import concourse.bass as bass
    import concourse.mybir as mybir
    import concourse.tile as tile
    from concourse import bacc

    f32 = mybir.dt.float32
    if mm_dtype_name is None:
        mm_dtype_name = os.environ.get("GRU_MM_DTYPE", "float32r")
    mm_dt = getattr(mybir.dt, mm_dtype_name)
    AF = mybir.ActivationFunctionType

    nc = bacc.Bacc("TRN2", target_bir_lowering=False)
    xT = nc.dram_tensor("xT", [D, s], f32, kind="ExternalInput")
    hT = nc.dram_tensor("hT", [D, s], f32, kind="ExternalInput")
    w_d = {n: nc.dram_tensor(n, [D, D], f32, kind="ExternalInput") for n in _WNAMES}
    b_d = {n: nc.dram_tensor(n, [D], f32, kind="ExternalInput") for n in _BNAMES}
    outT = nc.dram_tensor("outT", [D, s], f32, kind="ExternalOutput")

    nch = s // ch
    cast = mm_dt != f32
    # float32r is bit-identical to float32; allocate matmul operand tiles as
    # f32r and bitcast the fp32 views where engines need plain f32 semantics.
    f32r_mode = mm_dt == mybir.dt.float32r

    def md(ap):
        if ap.dtype == mm_dt:
            return ap
        return ap.bitcast(mm_dt) if cast else ap

    with tile.TileContext(nc) as tc:
        with (
            tc.tile_pool(name="const", bufs=1) as cpool,
            tc.tile_pool(name="inp", bufs=3) as ipool,
            tc.tile_pool(name="work", bufs=3) as wpool,
            tc.tile_pool(name="psum", bufs=1, space=bass.MemorySpace.PSUM) as ppool,
        ):
            # --- constants: weights [128, 256] x2 k-chunks each, biases [128, 2]
            w_sb = {}
            for n in _WNAMES:
                for k in range(2):
                    src = w_d[n][k * 128 : (k + 1) * 128, :]
                    if f32r_mode:
                        t = cpool.tile([128, D], mm_dt, tag=f"w_{n}_{k}")
                        nc.sync.dma_start(t[:], src.bitcast(mm_dt))
                    elif cast:
                        t0 = cpool.tile([128, D], f32, tag=f"wld_{n}_{k}")
                        nc.sync.dma_start(t0[:], src)
                        t = cpool.tile([128, D], mm_dt, tag=f"w_{n}_{k}")
                        nc.vector.tensor_copy(t[:], t0[:])
                    else:
                        t = cpool.tile([128, D], f32, tag=f"w_{n}_{k}")
                        nc.sync.dma_start(t[:], src)
                    w_sb[(n, k)] = t
            b_sb = {}
            for n in _BNAMES:
                t = cpool.tile([128, 2], f32, tag=f"b_{n}")
                nc.sync.dma_start(t[:], b_d[n].rearrange("(g p) -> p g", p=128))
                b_sb[n] = t

            def gate_psum(pool_tag, wn, un, rhs_w, rhs_u, g):
                """psum[{128},{ch}] = W[:,g].T @ rhs_w + U[:,g].T @ rhs_u."""
                p = ppool.tile([128, ch], f32, tag=pool_tag)
                gs = slice(g * 128, (g + 1) * 128)
                nc.tensor.matmul(p[:], md(w_sb[(wn, 0)][:, gs]), md(rhs_w[0][:]),
                                 start=True, stop=False)
                nc.tensor.matmul(p[:], md(w_sb[(wn, 1)][:, gs]), md(rhs_w[1][:]),
                                 start=False, stop=False)
                nc.tensor.matmul(p[:], md(w_sb[(un, 0)][:, gs]), md(rhs_u[0][:]),
                                 start=False, stop=False)
                nc.tensor.matmul(p[:], md(w_sb[(un, 1)][:, gs]), md(rhs_u[1][:]),
                                 start=False, stop=True)
                return p

            for c in range(nch):
                cols = slice(c * ch, (c + 1) * ch)
                # xt/ht: matmul-operand tiles; htf: f32 views of h for the
                # elementwise gate math.
                xt, ht, htf = [], [], []
                for k in range(2):
                    if f32r_mode:
                        tx = ipool.tile([128, ch], mm_dt, tag=f"x{k}")
                        nc.sync.dma_start(
                            tx[:], xT[k * 128 : (k + 1) * 128, cols].bitcast(mm_dt)
                        )
                        th = ipool.tile([128, ch], mm_dt, tag=f"h{k}")
                        nc.sync.dma_start(
                            th[:], hT[k * 128 : (k + 1) * 128, cols].bitcast(mm_dt)
                        )
                        xt.append(tx)
                        ht.append(th)
                        htf.append(th[:].bitcast(f32))
                    else:
                        tx = ipool.tile([128, ch], f32, tag=f"x{k}")
                        nc.sync.dma_start(tx[:], xT[k * 128 : (k + 1) * 128, cols])
                        th = ipool.tile([128, ch], f32, tag=f"h{k}")
                        nc.sync.dma_start(th[:], hT[k * 128 : (k + 1) * 128, cols])
                        htf.append(th[:])
                        if cast:
                            cx = ipool.tile([128, ch], mm_dt, tag=f"xc{k}")
                            nc.vector.tensor_copy(cx[:], tx[:])
                            chh = ipool.tile([128, ch], mm_dt, tag=f"hc{k}")
                            nc.vector.tensor_copy(chh[:], th[:])
                            xt.append(cx)
                            ht.append(chh)
                        else:
                            xt.append(tx)
                            ht.append(th)

                # reset gate -> r*h (needed before the candidate matmuls)
                rh = []
                for g in range(2):
                    pr = gate_psum(f"pr{g}", "Wr", "Ur", xt, ht, g)
                    rt = wpool.tile([128, ch], f32, tag=f"r{g}")
                    nc.scalar.activation(rt[:], pr[:], AF.Sigmoid,
                                         bias=b_sb["br"][:, g : g + 1])
                    t = wpool.tile([128, ch], mm_dt if cast else f32, tag=f"rh{g}")
                    nc.vector.tensor_mul(t[:], rt[:], htf[g])
                    rh.append(t)

                # update gate
                zt = []
                for g in range(2):
                    pz = gate_psum(f"pz{g}", "Wz", "Uz", xt, ht, g)
                    t = wpool.tile([128, ch], f32, tag=f"z{g}")
                    nc.scalar.activation(t[:], pz[:], AF.Sigmoid,
                                         bias=b_sb["bz"][:, g : g + 1])
                    zt.append(t)

                # candidate + combine + store
                for g in range(2):
                    ph = gate_psum(f"ph{g}", "Wh", "Uh", xt, rh, g)
                    hh = wpool.tile([128, ch], f32, tag=f"hh{g}")
                    nc.scalar.activation(hh[:], ph[:], AF.Tanh,
                                         bias=b_sb["bh"][:, g : g + 1])
                    d = wpool.tile([128, ch], f32, tag=f"d{g}")
                    nc.vector.tensor_sub(d[:], htf[g], hh[:])
                    m = wpool.tile([128, ch], f32, tag=f"m{g}")
                    nc.vector.tensor_mul(m[:], zt[g][:], d[:])
                    o = wpool.tile([128, ch], f32, tag=f"o{g}")
                    nc.vector.tensor_add(o[:], hh[:], m[:])
                    nc.sync.dma_start(outT[g * 128 : (g + 1) * 128, cols], o[:])

    nc.compile()
    return nc


_NC_CACHE = {}


def _get_nc():
    key = (S, os.environ.get("GRU_MM_DTYPE", "float32r"), CH)
    if key not in _NC_CACHE:
        _NC_CACHE[key] = build_nc(S, key[1], CH)
    return _NC_CACHE[key]


def _make_in_maps(inputs):
    f32 = np.float32
    x = np.asarray(inputs["x"], f32)
    h = np.asarray(inputs["h_t_1"], f32)
    consts = {n: np.ascontiguousarray(np.asarray(inputs[n], f32)) for n in _WNAMES}
    consts.update(
        {n: np.ascontiguousarray(np.asarray(inputs[n], f32)) for n in _BNAMES}
    )
    in_maps = []
    for c in range(N_CORES):
        sl = slice(c * S, (c + 1) * S)
        m = {
            "xT": np.ascontiguousarray(x[sl].T),
            "hT": np.ascontiguousarray(h[sl].T),
        }
        m.update(consts)
        in_maps.append(m)
    return in_maps


def run(inputs, trace=False):
    """Run on hardware; returns (h_t ndarray, BassKernelResults)."""
    from concourse.bass_utils import run_bass_kernel_spmd

    nc = _get_nc()
    in_maps = _make_in_maps(inputs)
    res = run_bass_kernel_spmd(nc, in_maps, list(range(N_CORES)), trace=trace)
    out = np.empty((B, D), np.float32)
    for c in range(N_CORES):
        out[c * S : (c + 1) * S] = res.results[c]["outT"].T
    return out, res


def kernel(**inputs):
    out, _ = run(inputs, trace=False)
    return (out, out)



# revision 2
# speedup vs baseline: 1.1581x; 1.1581x over previous
"""GRU cell on 8 Trainium2 NeuronCores.

Reference computation (B=65536, D=256):
    z = sigmoid(x@Wz + h@Uz + bz)
    r = sigmoid(x@Wr + h@Ur + br)
    h_hat = tanh(x@Wh + (r*h)@Uh + bh)
    h_t = z*h + (1-z)*h_hat  ; returns (h_t, h_t)

Strategy: data-parallel over the batch dim (8 shards of 8192 rows).

Per-core kernel (PE-roofline oriented):
- Host pre-packs every tensor into [128, free] bf16 layout:
  x/h shards as [128, 16 chunks x (2 hidden-halves x 512 batch)] so each
  chunk is one contiguous 256 KB DMA; weights as one [128, 12x256] pack
  (stationary operands need no on-chip transpose); biases [128, 6] f32.
- All six GEMMs run as bf16 matmuls (1 col/cycle, same PE rate as f32r)
  accumulating f32 in PSUM.  PSUM budget: P_r (2 banks) + P_z (2) +
  P_h double-buffered (4) = 8 banks.
- Software pipeline: iteration i issues PE groups r(i), z(i), h(i-1).
  The candidate-gate matmuls only consume rh = r*h from the *previous*
  iteration, so the PE stream never waits on the current chunk's
  ACT/DVE results -> no PE gaps -> HAM stays at K=8/8 (2.4 GHz).
- ACT applies bias+sigmoid/tanh straight out of PSUM (both in the same
  table set -> one table load); DVE does the 4 elementwise ops per
  chunk in bf16 2x mode.
- A burst of tiny warm-up matmuls (on the bias tile, issued behind the
  first 3 KB DMA) keeps the PE HAM activity window busy while weights
  and the first chunk stream in.
"""

import os
import sys

for _p in ("/opt/trn_rl_repo", "/root/.axon_site/_ro/trn_rl_repo"):
    if os.path.isdir(_p) and _p not in sys.path:
        sys.path.append(_p)

import numpy as np
import ml_dtypes

BF16 = ml_dtypes.bfloat16

B = 65536
D = 256
N_CORES = 8
S = B // N_CORES  # 8192 batch rows per core
CB = 512  # batch columns per chunk
NCH = S // CB  # 16 chunks
WARMUP_MMS = 24  # tiny PE warm-up matmuls before the real stream


def build_nc():
    import concourse.mybir as mybir
    import concourse.tile as tile
    from concourse import bacc

    f32 = mybir.dt.float32
    bf16 = mybir.dt.bfloat16
    AF = mybir.ActivationFunctionType

    nc = bacc.Bacc("TRN2", target_bir_lowering=False)
    # col layout: chunk c occupies [c*1024, (c+1)*1024): k-half major, batch minor
    xH = nc.dram_tensor("xH", [128, NCH * 2 * CB], bf16, kind="ExternalInput")
    hH = nc.dram_tensor("hH", [128, NCH * 2 * CB], bf16, kind="ExternalInput")
    # weight pack: 12 blocks of 256 cols: gate-major (r,z,h), within gate
    # [W k0, W k1, U k0, U k1]; block col m = output neuron
    wAll = nc.dram_tensor("wAll", [128, 12 * 256], bf16, kind="ExternalInput")
    # bias pack cols: [br g0, br g1, bz g0, bz g1, bh g0, bh g1]
    bAll = nc.dram_tensor("bAll", [128, 6], f32, kind="ExternalInput")
    oH = nc.dram_tensor("oH", [128, 2, S], bf16, kind="ExternalOutput")

    with tile.TileContext(nc) as tc:
        with (
            tc.tile_pool(name="const", bufs=1) as cpool,
            tc.tile_pool(name="inp", bufs=1) as ipool,
            tc.tile_pool(name="work", bufs=1) as wpool,
            tc.tile_pool(name="psum", bufs=1, space="PSUM") as ppool,
        ):
            bt = cpool.tile([128, 6], f32, tag="bias")
            nc.sync.dma_start(bt[:], bAll[:])

            # PE warm-up: tiny matmuls on the bias tile keep the HAM
            # activity window busy while the big DMAs land.  Results are
            # discarded (P_r is overwritten by a start=True group below).
            pwarm = ppool.tile([128, 2 * CB], f32, tag="p_r")
            for _ in range(WARMUP_MMS):
                nc.tensor.matmul(
                    pwarm[0:6, 0:6], bt[:, 0:6], bt[:, 0:6], start=True, stop=True
                )

            w_sb = {}
            for gi, gate in enumerate(("r", "z", "h")):
                t = cpool.tile([128, 4 * 256], bf16, tag=f"w_{gate}")
                nc.sync.dma_start(t[:], wAll[:, gi * 1024 : (gi + 1) * 1024])
                w_sb[gate] = t

            def mm_group(p, g, wt, rhs_w, rhs_u):
                """p[:, g*CB:(g+1)*CB] = W[:,g].T@rhs_w + U[:,g].T@rhs_u."""
                out = p[:, g * CB : (g + 1) * CB]
                for j, (w_i, rhs) in enumerate(
                    ((0, rhs_w), (1, rhs_w), (2, rhs_u), (3, rhs_u))
                ):
                    lhsT = wt[:, w_i * 256 + g * 128 : w_i * 256 + (g + 1) * 128]
                    k = w_i % 2
                    nc.tensor.matmul(
                        out,
                        lhsT,
                        rhs[:, k * CB : (k + 1) * CB],
                        start=(j == 0),
                        stop=(j == 3),
                    )

            xts, hts, rhs_t, zts = {}, {}, {}, {}

            def load_chunk(c):
                xt = ipool.tile([128, 2 * CB], bf16, tag="xt", bufs=4)
                nc.sync.dma_start(xt[:], xH[:, c * 1024 : (c + 1) * 1024])
                ht = ipool.tile([128, 2 * CB], bf16, tag="ht", bufs=4)
                nc.sync.dma_start(ht[:], hH[:, c * 1024 : (c + 1) * 1024])
                xts[c], hts[c] = xt, ht

            load_chunk(0)
            load_chunk(1)

            for i in range(NCH + 1):
                if i + 2 < NCH:
                    load_chunk(i + 2)

                # --- PE stream: r(i), z(i), h(i-1) ---
                if i < NCH:
                    xt, ht = xts[i], hts[i]
                    p_r = ppool.tile([128, 2 * CB], f32, tag="p_r")
                    mm_group(p_r, 0, w_sb["r"], xt, ht)
                    mm_group(p_r, 1, w_sb["r"], xt, ht)
                    p_z = ppool.tile([128, 2 * CB], f32, tag="p_z")
                    mm_group(p_z, 0, w_sb["z"], xt, ht)
                    mm_group(p_z, 1, w_sb["z"], xt, ht)
                if i >= 1:
                    c = i - 1
                    p_h = ppool.tile([128, 2 * CB], f32, tag="p_h", bufs=2)
                    mm_group(p_h, 0, w_sb["h"], xts[c], rhs_t[c])
                    mm_group(p_h, 1, w_sb["h"], xts[c], rhs_t[c])

                # --- ACT + DVE for r/z of chunk i ---
                if i < NCH:
                    rt = wpool.tile([128, 2 * CB], bf16, tag="rt", bufs=2)
                    nc.scalar.activation(
                        rt[:, 0:CB], p_r[:, 0:CB], AF.Sigmoid, bias=bt[:, 0:1]
                    )
                    nc.scalar.activation(
                        rt[:, CB:], p_r[:, CB:], AF.Sigmoid, bias=bt[:, 1:2]
                    )
                    zt = wpool.tile([128, 2 * CB], bf16, tag="zt", bufs=2)
                    nc.scalar.activation(
                        zt[:, 0:CB], p_z[:, 0:CB], AF.Sigmoid, bias=bt[:, 2:3]
                    )
                    nc.scalar.activation(
                        zt[:, CB:], p_z[:, CB:], AF.Sigmoid, bias=bt[:, 3:4]
                    )
                    zts[i] = zt
                    rh = wpool.tile([128, 2 * CB], bf16, tag="rh", bufs=2)
                    nc.vector.tensor_mul(rh[:], rt[:], ht[:])
                    rhs_t[i] = rh

                # --- ACT + DVE + store for candidate/combine of chunk i-1 ---
                if i >= 1:
                    c = i - 1
                    hh = wpool.tile([128, 2 * CB], bf16, tag="hh", bufs=2)
                    nc.scalar.activation(
                        hh[:, 0:CB], p_h[:, 0:CB], AF.Tanh, bias=bt[:, 4:5]
                    )
                    nc.scalar.activation(
                        hh[:, CB:], p_h[:, CB:], AF.Tanh, bias=bt[:, 5:6]
                    )
                    t1 = wpool.tile([128, 2 * CB], bf16, tag="t1", bufs=2)
                    nc.vector.tensor_sub(t1[:], hts[c][:], hh[:])
                    t2 = wpool.tile([128, 2 * CB], bf16, tag="t2", bufs=2)
                    nc.vector.tensor_mul(t2[:], zts[c][:], t1[:])
                    o = wpool.tile([128, 2 * CB], bf16, tag="o", bufs=2)
                    nc.vector.tensor_add(o[:], hh[:], t2[:])
                    nc.sync.dma_start(oH[:, :, c * CB : (c + 1) * CB], o[:])

    nc.compile()
    return nc


_NC_CACHE = {}


def _get_nc():
    if "nc" not in _NC_CACHE:
        _NC_CACHE["nc"] = build_nc()
    return _NC_CACHE["nc"]


def _pack_inputs(inputs):
    f32 = np.float32
    x = np.asarray(inputs["x"], f32)
    h = np.asarray(inputs["h_t_1"], f32)

    # weight pack [128, 12*256]: gate-major (r,z,h), per gate [Wk0,Wk1,Uk0,Uk1]
    blocks = []
    for wn, un in (("Wr", "Ur"), ("Wz", "Uz"), ("Wh", "Uh")):
        W = np.asarray(inputs[wn], f32)
        U = np.asarray(inputs[un], f32)
        blocks += [W[0:128], W[128:256], U[0:128], U[128:256]]
    wAll = np.ascontiguousarray(
        np.concatenate(blocks, axis=1).astype(BF16)
    )  # [128, 12*256]

    bAll = np.empty((128, 6), f32)
    for gi, bn in enumerate(("br", "bz", "bh")):
        b = np.asarray(inputs[bn], f32)
        bAll[:, 2 * gi] = b[0:128]
        bAll[:, 2 * gi + 1] = b[128:256]

    consts = {"wAll": wAll, "bAll": np.ascontiguousarray(bAll)}

    def pack_xh(a):
        # [S, 256] -> [128, NCH*2*CB]; col = c*1024 + k*512 + j
        a = a.astype(BF16).reshape(NCH, CB, 2, 128).transpose(3, 0, 2, 1)
        return np.ascontiguousarray(a.reshape(128, NCH * 2 * CB))

    in_maps = []
    for c in range(N_CORES):
        sl = slice(c * S, (c + 1) * S)
        m = {"xH": pack_xh(x[sl]), "hH": pack_xh(h[sl])}
        m.update(consts)
        in_maps.append(m)
    return in_maps


def run(inputs, trace=False):
    """Run on hardware; returns (h_t ndarray, BassKernelResults)."""
    from concourse.bass_utils import run_bass_kernel_spmd

    nc = _get_nc()
    in_maps = _pack_inputs(inputs)
    res = run_bass_kernel_spmd(nc, in_maps, list(range(N_CORES)), trace=trace)
    out = np.empty((B, D), np.float32)
    for c in range(N_CORES):
        oH = res.results[c]["oH"]  # [128, 2, S] bf16
        out[c * S : (c + 1) * S] = (
            oH.transpose(2, 1, 0).reshape(S, D).astype(np.float32)
        )
    return out, res


def kernel(**inputs):
    out, _ = run(inputs, trace=False)
    return (out, out)
